# revision 8
# baseline (speedup 1.0000x reference)
"""MoE-routed DIAYN discriminator kernel for 8 Trainium2 NeuronCores.

Reference semantics: x = concat([graph, state, next_state], -1); for each
row, run the 3-layer MLP of the LAST factor i<NF with graph[:, i]==1
(rows with no active factor output 0). The dense reference computes all
NF expert MLPs for every row; we instead route each row to exactly one
expert on the host, pack rows into 8 SPMD shards, and run one dense
per-expert MLP stream per core.

Sharding: rows are grouped by expert into BLK-row blocks. Every core
executes the same static "profile" of G runs (run g = prof[g] blocks);
each run uses one weight set, supplied per-core as data. A small host-side
search picks (G, prof) and an assignment of runs -> experts that covers
the actual per-expert block counts with minimal padding + weight traffic.

Device kernel (per run, per block, activations kept transposed [feat, row]):
  h1 = relu(W1^T x + b1); h2 = relu(W2^T h1 + b2); out = W3^T h2 + b3
matmuls run as fp32 bitcast to float32r (full-rate fp32 on the PE).
"""

import numpy as np
from ml_dtypes import bfloat16

import concourse.bass as bass
import concourse.mybir as mybir
from concourse import bacc
from concourse.tile import TileContext
from concourse.bass_utils import run_bass_kernel_spmd

NCORES = 8
BLK = 272  # rows per matmul block; <=512 (PSUM bank)

F32 = mybir.dt.float32
BF16 = mybir.dt.bfloat16

# Rough per-core cost weights for the plan search (ns).
_COST_BLOCK = int(152 * (BLK / 2.4 + 3))  # PE ns per block (152 matmuls)
_COST_RUN = 12_000  # partially-exposed weight-set DMA per extra run

_program_cache = {}


# ---------------------------------------------------------------- planning
def _compositions(total, parts):
    """Non-increasing positive integer compositions of `total` into `parts`."""
    if parts == 1:
        yield (total,)
        return
    for first in range((total + parts - 1) // parts, total - parts + 2):
        for rest in _compositions(total - first, parts - 1):
            if rest[0] <= first:
                yield (first,) + rest


def _try_assign(demands, prof):
    """Greedy cover of per-expert block demands by the 8x-replicated profile.

    demands: list of (n_blocks, expert) sorted desc. Returns dict
    run_size -> list of experts (8 entries per profile slot of that size,
    padding slots filled with the largest expert) or None if infeasible.
    """
    runs = sorted([t for t in prof for _ in range(NCORES)], reverse=True)
    used = []  # (size, expert)
    for n, e in demands:
        rem = n
        while rem > 0:
            if not runs:
                return None
            # largest run <= rem, else smallest run (minimal overshoot)
            pick = None
            for i, s in enumerate(runs):
                if s <= rem:
                    pick = i
                    break
            if pick is None:
                pick = len(runs) - 1
            s = runs.pop(pick)
            used.append((s, e))
            rem -= s
    pad_expert = demands[0][1]
    for s in runs:
        used.append((s, pad_expert))
    by_size = {}
    for s, e in used:
        by_size.setdefault(s, []).append(e)
    return by_size


def _make_plan(nblk):
    """nblk: per-expert block counts. Returns (prof, expert_of[core][g])."""
    demands = sorted(
        [(n, e) for e, n in enumerate(nblk) if n > 0], reverse=True
    )
    total = sum(n for n, _ in demands)
    mincap = (total + NCORES - 1) // NCORES
    best = None
    for G in range(1, 9):
        for cap in range(mincap, mincap + 6):
            for prof in _compositions(cap, G):
                a = _try_assign(demands, prof)
                if a is None:
                    continue
                cost = cap * _COST_BLOCK + G * _COST_RUN
                if best is None or cost < best[0]:
                    best = (cost, prof, a)
    assert best is not None, "no feasible run plan found"
    _, prof, by_size = best
    queues = {s: list(es) for s, es in by_size.items()}
    expert_of = [[None] * len(prof) for _ in range(NCORES)]
    for g, s in enumerate(prof):
        for core in range(NCORES):
            expert_of[core][g] = queues[s].pop(0)
    return list(prof), expert_of


# ---------------------------------------------------------------- device
def _build_program(prof, KO1, KO2, H, C, blk):
    """Build + compile the SPMD Bass program for a run profile."""
    key = (tuple(prof), KO1, KO2, H, C, blk)
    if key in _program_cache:
        return _program_cache[key]

    G = len(prof)
    NB = sum(prof)
    INP = KO1 * 128
    M1 = H // 128
    relu = mybir.ActivationFunctionType.Relu
    ident = mybir.ActivationFunctionType.Identity

    nc = bacc.Bacc("TRN2", target_bir_lowering=False, debug=False,
                   num_devices=NCORES)
    x_d = nc.dram_tensor("xb", [NB, 128, KO1, blk], BF16, kind="ExternalInput").ap()
    w1_d = nc.dram_tensor("w1", [G, 128, KO1, H], BF16, kind="ExternalInput").ap()
    w2_d = nc.dram_tensor("w2", [G, 128, KO2, H], BF16, kind="ExternalInput").ap()
    w3_d = nc.dram_tensor("w3", [G, 128, KO2, C], BF16, kind="ExternalInput").ap()
    b1_d = nc.dram_tensor("b1", [G, H], F32, kind="ExternalInput").ap()
    b2_d = nc.dram_tensor("b2", [G, H], F32, kind="ExternalInput").ap()
    b3_d = nc.dram_tensor("b3", [G, C], F32, kind="ExternalInput").ap()
    out_d = nc.dram_tensor("outb", [NB, C, blk], F32, kind="ExternalOutput").ap()

    runs = []
    for g, T in enumerate(prof):
        runs += [g] * T

    with TileContext(nc) as tc:
        with (
            tc.tile_pool(name="w", bufs=2) as wpool,
            tc.tile_pool(name="w0", bufs=1) as w0pool,
            tc.tile_pool(name="x", bufs=2) as xpool,
            tc.tile_pool(name="h1", bufs=3) as h1pool,
            tc.tile_pool(name="h2", bufs=1) as h2pool,
            tc.tile_pool(name="o", bufs=2) as opool,
            tc.tile_pool(name="ps", bufs=8, space="PSUM") as pspool,
        ):
            HH = H // 2

            def emit_weights(g, startup=False):
                # Startup (set 0, on the critical path): every W1 k-chunk
                # is split in half across the sync+gpsimd rings so chunks
                # arrive at ~2x queue rate just ahead of the k-outer L1
                # consumption; W2/W3 chunks follow on gpsimd. Steady sets
                # are prefetched ~4 blocks ahead, whole on the
                # then-idle sync ring; biases always ride scalar.
                if startup:
                    w1a, w1b = [], []
                    for k in range(KO1):
                        ta = w0pool.tile([128, HH], BF16, tag=f"w1k{k}a")
                        nc.sync.dma_start(ta[:], w1_d[g, :, k, :HH])
                        tb = w0pool.tile([128, HH], BF16, tag=f"w1k{k}b")
                        nc.gpsimd.dma_start(tb[:], w1_d[g, :, k, HH:])
                        w1a.append(ta)
                        w1b.append(tb)

                    def w1m(k, m):
                        if m < M1 // 2:
                            return w1a[k][:, m * 128:(m + 1) * 128]
                        mm = m - M1 // 2
                        return w1b[k][:, mm * 128:(mm + 1) * 128]
                else:
                    w1ch = []
                    for k in range(KO1):
                        wt = wpool.tile([128, H], BF16, tag=f"w1k{k}")
                        nc.sync.dma_start(wt[:], w1_d[g, :, k, :])
                        w1ch.append(wt)

                    def w1m(k, m):
                        return w1ch[k][:, m * 128:(m + 1) * 128]

                ring2 = nc.gpsimd if startup else nc.sync
                b1sb = wpool.tile([128, M1], F32, tag="b1")
                nc.scalar.dma_start(
                    b1sb[:], b1_d[g].rearrange("(m p) -> p m", p=128))
                b2sb = wpool.tile([128, M1], F32, tag="b2")
                nc.scalar.dma_start(
                    b2sb[:], b2_d[g].rearrange("(m p) -> p m", p=128))
                b3sb = wpool.tile([C, 1], F32, tag="b3")
                nc.scalar.dma_start(b3sb[:], b3_d[g][:, None])
                w2ch = []
                for k in range(KO2):
                    wt = wpool.tile([128, H], BF16, tag=f"w2k{k}")
                    ring2.dma_start(wt[:], w2_d[g, :, k, :])
                    w2ch.append(wt)
                w3sb = wpool.tile([128, KO2, C], BF16, tag="w3")
                ring2.dma_start(w3sb[:], w3_d[g])

                def w2(k):
                    return w2ch[k][:]

                return dict(w1m=w1m, w2=w2, w3=w3sb,
                            b1=b1sb, b2=b2sb, b3=b3sb)

            def emit_x(b):
                # x blocks ride the second HWDGE ring (scalar), parallel
                # to the weight stream on sync.
                xsb = xpool.tile([128, KO1, blk], BF16, tag="x")
                nc.scalar.dma_start(xsb[:], x_d[b])
                return xsb

            def emit_L1(W, xsb, kouter=False):
                h1sb = h1pool.tile([128, KO2, blk], BF16, tag="h1")
                if kouter:
                    # All 8 PSUM banks accumulate in parallel; each W1
                    # chunk is fully consumed on arrival (startup mode).
                    pss = [pspool.tile([128, blk], F32, tag="ps",
                                       name=f"ps_ko{m}")
                           for m in range(M1)]
                    for k in range(KO1):
                        for m in range(M1):
                            nc.tensor.matmul(
                                pss[m][:],
                                W["w1m"](k, m),
                                xsb[:, k, :],
                                start=(k == 0), stop=(k == KO1 - 1))
                    for m in range(M1):
                        nc.vector.tensor_scalar(
                            h1sb[:, m, :], pss[m][:], W["b1"][:, m:m + 1],
                            0.0, mybir.AluOpType.add, mybir.AluOpType.max)
                    return h1sb
                for m in range(M1):
                    ps = pspool.tile([128, blk], F32, tag="ps")
                    for k in range(KO1):
                        nc.tensor.matmul(
                            ps[:],
                            W["w1m"](k, m),
                            xsb[:, k, :],
                            start=(k == 0), stop=(k == KO1 - 1))
                    nc.vector.tensor_scalar(
                        h1sb[:, m, :], ps[:], W["b1"][:, m:m + 1], 0.0,
                        mybir.AluOpType.add, mybir.AluOpType.max)
                return h1sb

            def emit_L23(b, W, h1sb):
                h2sb = h2pool.tile([128, KO2, blk], BF16, tag="h2")
                for m in range(M1):
                    ps = pspool.tile([128, blk], F32, tag="ps")
                    for k in range(KO2):
                        nc.tensor.matmul(
                            ps[:],
                            W["w2"](k)[:, m * 128:(m + 1) * 128],
                            h1sb[:, k, :],
                            start=(k == 0), stop=(k == KO2 - 1))
                    nc.scalar.activation(
                        h2sb[:, m, :], ps[:], relu, bias=W["b2"][:, m:m + 1])
                ps3 = pspool.tile([128, blk], F32, tag="ps")
                for k in range(KO2):
                    nc.tensor.matmul(
                        ps3[:C, :],
                        W["w3"][:, k, :],
                        h2sb[:, k, :],
                        start=(k == 0), stop=(k == KO2 - 1))
                osb = opool.tile([C, blk], F32, tag="o")
                nc.scalar.activation(
                    osb[:], ps3[:C, :], ident, bias=W["b3"][:, 0:1])
                nc.gpsimd.dma_start(out_d[b], osb[:])

            # Software pipeline, depth 2: L1 of blocks b+1/b+2 are
            # emitted before L2/L3 of block b, so weight-set DMAs and
            # ACT latency never drain the PE (esp. during the initial
            # HBM-bound weight load).
            Ws = {}
            h1 = {}

            xpre = {}

            def emit_front(b):
                g = runs[b]
                if g not in Ws:
                    Ws[g] = emit_weights(g)
                h1[b] = emit_L1(Ws[g], xpre.pop(b) if b in xpre
                                else emit_x(b))

            # Startup: x0 chunks lead the scalar ring (k0 alone so the
            # first matmul can fire, then pairs), set-0 weights stream
            # split across sync+vector, x1/x2 follow on scalar, x3/x4 on
            # the otherwise-idle gpsimd ring. Block 0's L1 runs k-outer
            # so every half-chunk is consumed on arrival.
            g0 = runs[0]
            if prof[0] >= 3:
                xsb0 = xpool.tile([128, KO1, blk], BF16, tag="x",
                                  name="x0")
                nc.scalar.dma_start(xsb0[:, 0, :], x_d[0][:, 0, :])
                ks = 1
                while ks < KO1:
                    ke = min(ks + 2, KO1)
                    nc.scalar.dma_start(xsb0[:, ks:ke, :],
                                        x_d[0][:, ks:ke, :])
                    ks = ke
                Ws[g0] = emit_weights(g0, startup=True)
                xs1 = emit_x(1)
                xs2 = emit_x(2)
                for bb in (3, 4):
                    if bb < NB:
                        xp = xpool.tile([128, KO1, blk], BF16, tag="x",
                                        name=f"xpre{bb}")
                        nc.scalar.dma_start(xp[:], x_d[bb])
                        xpre[bb] = xp
                h1[0] = emit_L1(Ws[g0], xsb0, kouter=True)
                h1[1] = emit_L1(Ws[g0], xs1)
                h1[2] = emit_L1(Ws[g0], xs2)
                emitted = 2
            else:
                emit_front(0)
                emitted = 0
            for b in range(NB):
                for nxt in range(emitted + 1, min(b + 3, NB)):
                    emit_front(nxt)
                    emitted = nxt
                if b + 4 < NB and runs[b + 4] not in Ws:
                    Ws[runs[b + 4]] = emit_weights(runs[b + 4])
                emit_L23(b, Ws[runs[b]], h1.pop(b))

    nc.compile()
    _program_cache[key] = nc
    return nc


# ---------------------------------------------------------------- host
def _execute(inputs, trace=False, trace_cores=None):
    graph = np.ascontiguousarray(inputs["graph"], dtype=np.float32)
    state = np.ascontiguousarray(inputs["state"], dtype=np.float32)
    next_state = np.ascontiguousarray(inputs["next_state"], dtype=np.float32)
    W1 = np.ascontiguousarray(inputs["W1"], dtype=np.float32)
    b1 = np.ascontiguousarray(inputs["b1"], dtype=np.float32)
    W2 = np.ascontiguousarray(inputs["W2"], dtype=np.float32)
    b2 = np.ascontiguousarray(inputs["b2"], dtype=np.float32)
    W3 = np.ascontiguousarray(inputs["W3"], dtype=np.float32)
    b3 = np.ascontiguousarray(inputs["b3"], dtype=np.float32)

    B = graph.shape[0]
    NF, IN, H = W1.shape
    C = W3.shape[2]
    assert IN == graph.shape[1] + state.shape[1] + next_state.shape[1]
    assert H % 128 == 0 and C <= 128
    INP = ((IN + 127) // 128) * 128
    KO1 = INP // 128

    out_full = np.zeros((B, C), dtype=np.float32)

    # --- route: last active factor per row
    mask = graph[:, :NF] == 1.0
    active = mask.any(axis=1)
    last = (NF - 1) - np.argmax(mask[:, ::-1], axis=1)
    if not active.any():
        return (out_full, None) if trace else out_full

    rows_by_e = [np.nonzero(active & (last == e))[0] for e in range(NF)]
    nblk = [(len(r) + BLK - 1) // BLK for r in rows_by_e]
    prof, expert_of = _make_plan(nblk)
    G, NB = len(prof), sum(prof)

    # --- pack rows into per-core block slots
    # rowmap[core] : int32 [NB, BLK], original row id or -1 (pad)
    rowmap = [np.full((NB, BLK), -1, dtype=np.int64) for _ in range(NCORES)]
    off = np.cumsum([0] + prof)  # run g occupies blocks [off[g], off[g+1])
    slots_by_e = {}
    for core in range(NCORES):
        for g in range(G):
            slots_by_e.setdefault(expert_of[core][g], []).append((core, g))
    for e in range(NF):
        rows = rows_by_e[e]
        if len(rows) == 0:
            continue
        pos = 0
        for core, g in slots_by_e.get(e, []):
            cap = prof[g] * BLK
            take = min(cap, len(rows) - pos)
            if take <= 0:
                break
            flat = rowmap[core][off[g]:off[g + 1]].reshape(-1)
            flat[:take] = rows[pos:pos + take]
            pos += take
        assert pos == len(rows), f"expert {e} rows not fully packed"

    # --- build per-core inputs
    x = np.concatenate([graph, state, next_state], axis=1)  # [B, IN]
    if INP != IN:
        x = np.concatenate([x, np.zeros((B, INP - IN), np.float32)], axis=1)
    xpad = np.concatenate([x, np.zeros((1, INP), np.float32)], axis=0)
    W1p = np.zeros((NF, INP, H), np.float32)
    W1p[:, :IN] = W1

    # Partition-major device layouts: [.., 128, KO, free] so every DMA
    # line is one contiguous 10-20KB run per partition.
    KO2 = H // 128
    W1pm = np.ascontiguousarray(
        W1p.reshape(NF, KO1, 128, H).transpose(0, 2, 1, 3)).astype(bfloat16)
    W2pm = np.ascontiguousarray(
        W2.reshape(NF, KO2, 128, H).transpose(0, 2, 1, 3)).astype(bfloat16)
    W3pm = np.ascontiguousarray(
        W3.reshape(NF, KO2, 128, C).transpose(0, 2, 1, 3)).astype(bfloat16)
    in_maps = []
    for core in range(NCORES):
        xb = xpad[rowmap[core].reshape(-1)]  # [NB*BLK, INP]; -1 -> zero row
        xb = np.ascontiguousarray(
            xb.reshape(NB, BLK, KO1, 128).transpose(0, 3, 2, 1)).astype(bfloat16)
        es = expert_of[core]
        in_maps.append({
            "xb": xb,
            "w1": W1pm[es],
            "w2": W2pm[es],
            "w3": W3pm[es],
            "b1": np.ascontiguousarray(b1[es]),
            "b2": np.ascontiguousarray(b2[es]),
            "b3": np.ascontiguousarray(b3[es]),
        })

    nc = _build_program(prof, KO1, KO2, H, C, BLK)
    kwargs = {}
    if trace:
        kwargs = dict(trace=True,
                      trace_cores=trace_cores or list(range(NCORES)))
    res = run_bass_kernel_spmd(nc, in_maps, list(range(NCORES)), **kwargs)

    # --- scatter back
    for core in range(NCORES):
        ob = np.asarray(res.results[core]["outb"])  # [NB, C, BLK]
        rows = ob.transpose(0, 2, 1).reshape(NB * BLK, C)
        ids = rowmap[core].reshape(-1)
        valid = ids >= 0
        out_full[ids[valid]] = rows[valid]

    return (out_full, res) if trace else out_full


def kernel(**inputs):
    return _execute(inputs)



# revision 11
# speedup vs baseline: 1.0106x; 1.0106x over previous
"""MoE-routed DIAYN discriminator kernel for 8 Trainium2 NeuronCores.

Reference semantics: x = concat([graph, state, next_state], -1); for each
row, run the 3-layer MLP of the LAST factor i<NF with graph[:, i]==1
(rows with no active factor output 0). The dense reference computes all
NF expert MLPs for every row; we instead route each row to exactly one
expert on the host, pack rows into per-expert blocks, and run one dense
per-expert MLP stream per core.

Sharding: every core executes the same static profile of G runs; run g
is T_g blocks of S_g rows and uses one weight set, supplied per-core as
data. A host-side search picks the profile (variable block sizes: a big
first run hides the HBM-bound initial weight load behind longer matmuls,
a small tail run trims row padding) and an assignment of (core, run)
slots -> experts covering the actual per-expert row counts.

Device kernel (per run, per block, activations kept transposed
[feat, row], bf16 operands, fp32 PSUM accumulation):
  h1 = relu(W1^T x + b1); h2 = relu(W2^T h1 + b2); out = W3^T h2 + b3
"""

import numpy as np
from ml_dtypes import bfloat16

import concourse.bass as bass
import concourse.mybir as mybir
from concourse import bacc
from concourse.tile import TileContext
from concourse.bass_utils import run_bass_kernel_spmd

NCORES = 8

F32 = mybir.dt.float32
BF16 = mybir.dt.bfloat16

_program_cache = {}


# ---------------------------------------------------------------- planning
def _mm_ns(s):
    """Measured per-matmul ns for an s-row moving dim (bf16, 2.4GHz)."""
    return 0.4167 * s + 2.7


def _blk_ns(s):
    """Per-block PE ns: 80 L1 + 64 L2 + 8 L3 matmuls."""
    return 152 * _mm_ns(s)


def _startup_gap(s0):
    """Exposed PE idle while set-0 W1 streams in: 9 chunk arrivals at
    ~1550ns vs k-outer consumption of 8 matmuls per chunk."""
    return 9.0 * max(0.0, 1550.0 - 8.0 * _mm_ns(s0))


def _try_assign(demands, slots):
    """Greedy cover of per-expert row demands by slot capacities.

    demands: [(rows, expert)] sorted desc. slots: list of caps (8 per
    profile run). Returns {slot_index: expert} covering all demands or
    None. Leftover slots get expert of the largest demand (all-pad).
    """
    order = sorted(range(len(slots)), key=lambda i: -slots[i])
    free = [True] * len(slots)
    assign = {}
    for rows, e in demands:
        rem = rows
        while rem > 0:
            pick = None
            # largest free slot <= rem
            for i in order:
                if free[i] and slots[i] <= rem:
                    pick = i
                    break
            if pick is None:
                # smallest free slot (> rem): minimal overshoot
                for i in reversed(order):
                    if free[i]:
                        pick = i
                        break
            if pick is None:
                return None
            free[pick] = False
            assign[pick] = e
            rem -= slots[pick]
    pad = demands[0][1]
    for i in range(len(slots)):
        if free[i]:
            assign[i] = pad
    return assign


def _make_plan(rows_by_e):
    """rows_by_e: per-expert row counts. Returns (prof, expert_of) with
    prof = [(T_g, S_g)] and expert_of[core][g] = expert index."""
    demands = sorted(
        [(n, e) for e, n in enumerate(rows_by_e) if n > 0], reverse=True
    )
    total = sum(n for n, _ in demands)
    percore = (total + NCORES - 1) // NCORES

    S0S = [512, 448, 384, 320, 272]
    T0S = [3, 4, 5, 6]
    SS = [512, 448, 384, 320, 272, 240, 208, 176, 144, 112, 80]
    TS = [1, 2, 3, 4, 5]
    from itertools import combinations_with_replacement as cwr

    rest_specs = [(t, s) for t in TS for s in SS]
    best = None

    def consider(prof):
        nonlocal best
        cap = sum(t * s for t, s in prof)
        if cap < percore or cap > percore + 700:
            return
        cost = (_startup_gap(prof[0][1])
                + sum(t * _blk_ns(s) for t, s in prof)
                + (len(prof) - 1) * 1500.0 + 0.3 * prof[-1][1])
        if best is not None and cost >= best[0]:
            return
        slots = [t * s for t, s in prof for _ in range(NCORES)]
        assign = _try_assign(demands, slots)
        if assign is None:
            return
        best = (cost, list(prof), assign)

    for t0 in T0S:
        for s0 in S0S:
            consider([(t0, s0)])
            for nrest in (1, 2):
                for rest in cwr(rest_specs, nrest):
                    consider([(t0, s0)] + list(rest))

    assert best is not None, "no feasible run plan found"
    _, prof, assign = best
    G = len(prof)
    expert_of = [[None] * G for _ in range(NCORES)]
    for idx, e in assign.items():
        g, core = divmod(idx, NCORES)
        expert_of[core][g] = e
    return prof, expert_of


# ---------------------------------------------------------------- device
def _build_program(prof, KO1, KO2, H, C):
    """Build + compile the SPMD Bass program for a run profile."""
    key = (tuple(prof), KO1, KO2, H, C)
    if key in _program_cache:
        return _program_cache[key]

    G = len(prof)
    M1 = H // 128
    relu = mybir.ActivationFunctionType.Relu
    ident = mybir.ActivationFunctionType.Identity

    nc = bacc.Bacc("TRN2", target_bir_lowering=False, debug=False,
                   num_devices=NCORES)
    x_d = [nc.dram_tensor(f"xb{g}", [T, 128, KO1, S], BF16,
                          kind="ExternalInput").ap()
           for g, (T, S) in enumerate(prof)]
    w1_d = nc.dram_tensor("w1", [G, 128, KO1, H], BF16,
                          kind="ExternalInput").ap()
    w2_d = nc.dram_tensor("w2", [G, 128, KO2, H], BF16,
                          kind="ExternalInput").ap()
    w3_d = nc.dram_tensor("w3", [G, 128, KO2, C], BF16,
                          kind="ExternalInput").ap()
    b1_d = nc.dram_tensor("b1", [G, H], F32, kind="ExternalInput").ap()
    b2_d = nc.dram_tensor("b2", [G, H], F32, kind="ExternalInput").ap()
    b3_d = nc.dram_tensor("b3", [G, C], F32, kind="ExternalInput").ap()
    out_d = [nc.dram_tensor(f"outb{g}", [T, C, S], F32,
                            kind="ExternalOutput").ap()
             for g, (T, S) in enumerate(prof)]

    # flat block list: (g, t) in execution order
    blocks = [(g, t) for g, (T, S) in enumerate(prof) for t in range(T)]
    NB = len(blocks)

    with TileContext(nc) as tc:
        with (
            tc.tile_pool(name="w", bufs=2) as wpool,
            tc.tile_pool(name="x", bufs=3) as xpool,
            tc.tile_pool(name="h1", bufs=3) as h1pool,
            tc.tile_pool(name="h2", bufs=1) as h2pool,
            tc.tile_pool(name="o", bufs=2) as opool,
            tc.tile_pool(name="ps", bufs=8, space="PSUM") as pspool,
        ):
            def emit_weights(g, startup=False):
                # W1 k-chunks on sync (the critical startup stream; also
                # carries later sets, prefetched ~4 blocks ahead). W2/W3
                # ride gpsimd for the startup set (sync must stay free
                # for W1), sync otherwise. Biases always on scalar.
                w1ch = []
                for k in range(KO1):
                    wt = wpool.tile([128, H], BF16, tag=f"w1k{k}")
                    nc.sync.dma_start(wt[:], w1_d[g, :, k, :])
                    w1ch.append(wt)

                ring2 = nc.gpsimd if startup else nc.sync
                b1sb = wpool.tile([128, M1], F32, tag="b1")
                nc.scalar.dma_start(
                    b1sb[:], b1_d[g].rearrange("(m p) -> p m", p=128))
                b2sb = wpool.tile([128, M1], F32, tag="b2")
                nc.scalar.dma_start(
                    b2sb[:], b2_d[g].rearrange("(m p) -> p m", p=128))
                b3sb = wpool.tile([C, 1], F32, tag="b3")
                nc.scalar.dma_start(b3sb[:], b3_d[g][:, None])
                w2ch = []
                for k in range(KO2):
                    wt = wpool.tile([128, H], BF16, tag=f"w2k{k}")
                    ring2.dma_start(wt[:], w2_d[g, :, k, :])
                    w2ch.append(wt)
                w3sb = wpool.tile([128, KO2, C], BF16, tag="w3")
                ring2.dma_start(w3sb[:], w3_d[g])

                return dict(
                    w1m=lambda k, m: w1ch[k][:, m * 128:(m + 1) * 128],
                    w2=lambda k: w2ch[k][:], w3=w3sb,
                    b1=b1sb, b2=b2sb, b3=b3sb)

            def emit_x(b):
                g, t = blocks[b]
                S = prof[g][1]
                xsb = xpool.tile([128, KO1, S], BF16, tag="x")
                nc.scalar.dma_start(xsb[:], x_d[g][t])
                return xsb

            def emit_L1(b, W, xsb, kouter=False):
                g, _ = blocks[b]
                S = prof[g][1]
                h1sb = h1pool.tile([128, KO2, S], BF16, tag="h1")
                if kouter:
                    # All 8 PSUM banks accumulate in parallel; each W1
                    # chunk is fully consumed on arrival (startup mode).
                    pss = [pspool.tile([128, S], F32, tag="ps",
                                       name=f"ps_ko{m}")
                           for m in range(M1)]
                    for k in range(KO1):
                        for m in range(M1):
                            nc.tensor.matmul(
                                pss[m][:], W["w1m"](k, m), xsb[:, k, :],
                                start=(k == 0), stop=(k == KO1 - 1))
                    for m in range(M1):
                        nc.vector.tensor_scalar(
                            h1sb[:, m, :], pss[m][:], W["b1"][:, m:m + 1],
                            0.0, mybir.AluOpType.add, mybir.AluOpType.max)
                    return h1sb
                for m in range(M1):
                    ps = pspool.tile([128, S], F32, tag="ps",
                                     name=f"ps_{b}_{m}")
                    for k in range(KO1):
                        nc.tensor.matmul(
                            ps[:], W["w1m"](k, m), xsb[:, k, :],
                            start=(k == 0), stop=(k == KO1 - 1))
                    nc.vector.tensor_scalar(
                        h1sb[:, m, :], ps[:], W["b1"][:, m:m + 1], 0.0,
                        mybir.AluOpType.add, mybir.AluOpType.max)
                return h1sb

            def emit_L23(b, W, h1sb):
                g, t = blocks[b]
                S = prof[g][1]
                h2sb = h2pool.tile([128, KO2, S], BF16, tag="h2")
                for m in range(M1):
                    ps = pspool.tile([128, S], F32, tag="ps",
                                     name=f"ps2_{b}_{m}")
                    for k in range(KO2):
                        nc.tensor.matmul(
                            ps[:], W["w2"](k)[:, m * 128:(m + 1) * 128],
                            h1sb[:, k, :],
                            start=(k == 0), stop=(k == KO2 - 1))
                    nc.scalar.activation(
                        h2sb[:, m, :], ps[:], relu, bias=W["b2"][:, m:m + 1])
                ps3 = pspool.tile([128, S], F32, tag="ps",
                                  name=f"ps3_{b}")
                for k in range(KO2):
                    nc.tensor.matmul(
                        ps3[:C, :], W["w3"][:, k, :], h2sb[:, k, :],
                        start=(k == 0), stop=(k == KO2 - 1))
                osb = opool.tile([C, S], F32, tag="o")
                nc.scalar.activation(
                    osb[:], ps3[:C, :], ident, bias=W["b3"][:, 0:1])
                nc.gpsimd.dma_start(out_d[g][t], osb[:])

            # Software pipeline, depth 2: L1 of blocks b+1/b+2 are
            # emitted before L2/L3 of block b, so weight-set DMAs and
            # ACT latency never drain the PE.
            Ws = {}
            h1 = {}
            xpre = {}

            def emit_front(b):
                g = blocks[b][0]
                if g not in Ws:
                    Ws[g] = emit_weights(g)
                h1[b] = emit_L1(b, Ws[g], xpre.pop(b) if b in xpre
                                else emit_x(b))

            # Startup: x0 chunks lead the scalar ring (k0 alone so the
            # first matmul can fire, then pairs), set-0 W1 streams on
            # sync, W2/W3 on gpsimd, x1..x4 follow on scalar. Block 0's
            # L1 runs k-outer so every chunk is consumed on arrival.
            g0 = blocks[0][0]
            T0, S0 = prof[0]
            if T0 >= 3:
                xsb0 = xpool.tile([128, KO1, S0], BF16, tag="x",
                                  name="x0")
                nc.scalar.dma_start(xsb0[:, 0, :], x_d[0][0, :, 0, :])
                ks = 1
                while ks < KO1:
                    ke = min(ks + 2, KO1)
                    nc.scalar.dma_start(xsb0[:, ks:ke, :],
                                        x_d[0][0, :, ks:ke, :])
                    ks = ke
                Ws[g0] = emit_weights(g0, startup=True)
                xs1 = emit_x(1)
                xs2 = emit_x(2)
                for bb in (3, 4):
                    if bb < NB:
                        g, t = blocks[bb]
                        S = prof[g][1]
                        xp = xpool.tile([128, KO1, S], BF16,
                                        tag="x", name=f"xpre{bb}")
                        nc.scalar.dma_start(xp[:], x_d[g][t])
                        xpre[bb] = xp
                h1[0] = emit_L1(0, Ws[g0], xsb0, kouter=True)
                h1[1] = emit_L1(1, Ws[g0], xs1)
                h1[2] = emit_L1(2, Ws[g0], xs2)
                emitted = 2
            else:
                emit_front(0)
                emitted = 0
            for b in range(NB):
                for nxt in range(emitted + 1, min(b + 3, NB)):
                    emit_front(nxt)
                    emitted = nxt
                if b + 4 < NB and blocks[b + 4][0] not in Ws:
                    Ws[blocks[b + 4][0]] = emit_weights(blocks[b + 4][0])
                emit_L23(b, Ws[blocks[b][0]], h1.pop(b))

    nc.compile()
    _program_cache[key] = nc
    return nc


# ---------------------------------------------------------------- host
def _execute(inputs, trace=False, trace_cores=None):
    graph = np.ascontiguousarray(inputs["graph"], dtype=np.float32)
    state = np.ascontiguousarray(inputs["state"], dtype=np.float32)
    next_state = np.ascontiguousarray(inputs["next_state"], dtype=np.float32)
    W1 = np.ascontiguousarray(inputs["W1"], dtype=np.float32)
    b1 = np.ascontiguousarray(inputs["b1"], dtype=np.float32)
    W2 = np.ascontiguousarray(inputs["W2"], dtype=np.float32)
    b2 = np.ascontiguousarray(inputs["b2"], dtype=np.float32)
    W3 = np.ascontiguousarray(inputs["W3"], dtype=np.float32)
    b3 = np.ascontiguousarray(inputs["b3"], dtype=np.float32)

    B = graph.shape[0]
    NF, IN, H = W1.shape
    C = W3.shape[2]
    assert IN == graph.shape[1] + state.shape[1] + next_state.shape[1]
    assert H % 128 == 0 and C <= 128
    INP = ((IN + 127) // 128) * 128
    KO1 = INP // 128
    KO2 = H // 128

    out_full = np.zeros((B, C), dtype=np.float32)

    # --- route: last active factor per row
    mask = graph[:, :NF] == 1.0
    active = mask.any(axis=1)
    last = (NF - 1) - np.argmax(mask[:, ::-1], axis=1)
    if not active.any():
        return (out_full, None) if trace else out_full

    rows_by_e = [np.nonzero(active & (last == e))[0] for e in range(NF)]
    prof, expert_of = _make_plan([len(r) for r in rows_by_e])
    G = len(prof)

    # --- pack rows into per-core slot blocks
    # rowmap[core][g] : int64 [T_g, S_g], original row id or -1 (pad)
    rowmap = [[np.full((T, S), -1, dtype=np.int64) for (T, S) in prof]
              for _ in range(NCORES)]
    slots_by_e = {}
    for core in range(NCORES):
        for g in range(G):
            slots_by_e.setdefault(expert_of[core][g], []).append((core, g))
    for e in range(NF):
        rows = rows_by_e[e]
        if len(rows) == 0:
            continue
        pos = 0
        for core, g in slots_by_e.get(e, []):
            T, S = prof[g]
            take = min(T * S, len(rows) - pos)
            if take <= 0:
                break
            flat = rowmap[core][g].reshape(-1)
            flat[:take] = rows[pos:pos + take]
            pos += take
        assert pos == len(rows), f"expert {e} rows not fully packed"

    # --- build per-core inputs
    x = np.concatenate([graph, state, next_state], axis=1)  # [B, IN]
    if INP != IN:
        x = np.concatenate([x, np.zeros((B, INP - IN), np.float32)], axis=1)
    xpad = np.concatenate([x, np.zeros((1, INP), np.float32)], axis=0)
    W1p = np.zeros((NF, INP, H), np.float32)
    W1p[:, :IN] = W1

    # Partition-major device layouts: [.., 128, KO, free] so every DMA
    # line is one contiguous run per partition.
    W1pm = np.ascontiguousarray(
        W1p.reshape(NF, KO1, 128, H).transpose(0, 2, 1, 3)).astype(bfloat16)
    W2pm = np.ascontiguousarray(
        W2.reshape(NF, KO2, 128, H).transpose(0, 2, 1, 3)).astype(bfloat16)
    W3pm = np.ascontiguousarray(
        W3.reshape(NF, KO2, 128, C).transpose(0, 2, 1, 3)).astype(bfloat16)
    in_maps = []
    for core in range(NCORES):
        es = expert_of[core]
        im = {
            "w1": W1pm[es],
            "w2": W2pm[es],
            "w3": W3pm[es],
            "b1": np.ascontiguousarray(b1[es]),
            "b2": np.ascontiguousarray(b2[es]),
            "b3": np.ascontiguousarray(b3[es]),
        }
        for g, (T, S) in enumerate(prof):
            xb = xpad[rowmap[core][g].reshape(-1)]  # [T*S, INP]; -1 -> 0row
            im[f"xb{g}"] = np.ascontiguousarray(
                xb.reshape(T, S, KO1, 128).transpose(0, 3, 2, 1)
            ).astype(bfloat16)
        in_maps.append(im)

    nc = _build_program(tuple(prof), KO1, KO2, H, C)
    kwargs = {}
    if trace:
        kwargs = dict(trace=True,
                      trace_cores=trace_cores or list(range(NCORES)))
    res = run_bass_kernel_spmd(nc, in_maps, list(range(NCORES)), **kwargs)

    # --- scatter back
    for core in range(NCORES):
        for g, (T, S) in enumerate(prof):
            ob = np.asarray(res.results[core][f"outb{g}"])  # [T, C, S]
            rows = ob.transpose(0, 2, 1).reshape(T * S, C)
            ids = rowmap[core][g].reshape(-1)
            valid = ids >= 0
            out_full[ids[valid]] = rows[valid]

    return (out_full, res) if trace else out_full


def kernel(**inputs):
    return _execute(inputs)


# revision 12
# speedup vs baseline: 1.0598x; 1.0487x over previous
"""MoE-routed DIAYN discriminator kernel for 8 Trainium2 NeuronCores.

Reference semantics: x = concat([graph, state, next_state], -1); for each
row, run the 3-layer MLP of the LAST factor i<NF with graph[:, i]==1
(rows with no active factor output 0). The dense reference computes all
NF expert MLPs for every row; we instead route each row to exactly one
expert on the host, pack rows into per-expert blocks, and run one dense
per-expert MLP stream per core.

Sharding: every core executes the same static profile of G runs; run g
is T_g blocks of S_g rows and uses one weight set, supplied per-core as
data. A host-side search picks the profile (variable block sizes: a big
first run hides the HBM-bound initial weight load behind longer matmuls,
a small tail run trims row padding) and an assignment of (core, run)
slots -> experts covering the actual per-expert row counts.

Device kernel (per run, per block, activations kept transposed
[feat, row], bf16 operands, fp32 PSUM accumulation):
  h1 = relu(W1^T x + b1); h2 = relu(W2^T h1 + b2); out = W3^T h2 + b3
"""

import numpy as np
from ml_dtypes import bfloat16

import concourse.bass as bass
import concourse.mybir as mybir
from concourse import bacc
from concourse.tile import TileContext
from concourse.bass_utils import run_bass_kernel_spmd

NCORES = 8

F32 = mybir.dt.float32
BF16 = mybir.dt.bfloat16

_program_cache = {}


# ---------------------------------------------------------------- planning
def _mm_ns(s):
    """Measured per-matmul ns for an s-row moving dim (bf16, 2.4GHz)."""
    return 0.4167 * s + 2.7


def _blk_ns(s):
    """Per-block PE ns: 80 L1 + 64 L2 + 8 L3 matmuls."""
    return 152 * _mm_ns(s)


def _startup_gap(s0):
    """Exposed PE idle while set-0 W1 streams in: 9 chunk arrivals at
    ~1550ns vs k-outer consumption of 8 matmuls per chunk."""
    return 9.0 * max(0.0, 1550.0 - 8.0 * _mm_ns(s0))


def _try_assign(demands, slots):
    """Greedy cover of per-expert row demands by slot capacities.

    demands: [(rows, expert)] sorted desc. slots: list of caps (8 per
    profile run). Returns {slot_index: expert} covering all demands or
    None. Leftover slots get expert of the largest demand (all-pad).
    """
    order = sorted(range(len(slots)), key=lambda i: -slots[i])
    free = [True] * len(slots)
    assign = {}
    for rows, e in demands:
        rem = rows
        while rem > 0:
            pick = None
            # largest free slot <= rem
            for i in order:
                if free[i] and slots[i] <= rem:
                    pick = i
                    break
            if pick is None:
                # smallest free slot (> rem): minimal overshoot
                for i in reversed(order):
                    if free[i]:
                        pick = i
                        break
            if pick is None:
                return None
            free[pick] = False
            assign[pick] = e
            rem -= slots[pick]
    pad = demands[0][1]
    for i in range(len(slots)):
        if free[i]:
            assign[i] = pad
    return assign


def _make_plan(rows_by_e):
    """rows_by_e: per-expert row counts. Returns (prof, expert_of) with
    prof = [(T_g, S_g)] and expert_of[core][g] = expert index."""
    demands = sorted(
        [(n, e) for e, n in enumerate(rows_by_e) if n > 0], reverse=True
    )
    total = sum(n for n, _ in demands)
    percore = (total + NCORES - 1) // NCORES

    S0S = [512, 448, 384, 320, 272]
    T0S = [3, 4, 5, 6]
    SS = [512, 448, 384, 320, 272, 240, 208, 176, 144, 112, 80]
    TS = [1, 2, 3, 4, 5]
    from itertools import combinations_with_replacement as cwr

    rest_specs = [(t, s) for t in TS for s in SS]
    best = None

    def consider(prof):
        nonlocal best
        cap = sum(t * s for t, s in prof)
        if cap < percore or cap > percore + 700:
            return
        cost = (_startup_gap(prof[0][1])
                + sum(t * _blk_ns(s) for t, s in prof)
                + (len(prof) - 1) * 1500.0 + 0.3 * prof[-1][1])
        if best is not None and cost >= best[0]:
            return
        slots = [t * s for t, s in prof for _ in range(NCORES)]
        assign = _try_assign(demands, slots)
        if assign is None:
            return
        best = (cost, list(prof), assign)

    for t0 in T0S:
        for s0 in S0S:
            consider([(t0, s0)])
            for nrest in (1, 2):
                for rest in cwr(rest_specs, nrest):
                    consider([(t0, s0)] + list(rest))

    assert best is not None, "no feasible run plan found"
    _, prof, assign = best
    G = len(prof)
    expert_of = [[None] * G for _ in range(NCORES)]
    for idx, e in assign.items():
        g, core = divmod(idx, NCORES)
        expert_of[core][g] = e
    return prof, expert_of


# ---------------------------------------------------------------- device
def _build_program(prof, KO1, KO2, H, C):
    """Build + compile the SPMD Bass program for a run profile."""
    key = (tuple(prof), KO1, KO2, H, C)
    if key in _program_cache:
        return _program_cache[key]

    G = len(prof)
    M1 = H // 128
    relu = mybir.ActivationFunctionType.Relu
    ident = mybir.ActivationFunctionType.Identity

    nc = bacc.Bacc("TRN2", target_bir_lowering=False, debug=False,
                   num_devices=NCORES)
    x_d = [nc.dram_tensor(f"xb{g}", [T, 128, KO1, S], BF16,
                          kind="ExternalInput").ap()
           for g, (T, S) in enumerate(prof)]
    w1_d = nc.dram_tensor("w1", [G, 128, KO1, H], BF16,
                          kind="ExternalInput").ap()
    w2_d = nc.dram_tensor("w2", [G, 128, KO2, H], BF16,
                          kind="ExternalInput").ap()
    w3_d = nc.dram_tensor("w3", [G, 128, KO2, C], BF16,
                          kind="ExternalInput").ap()
    b1_d = nc.dram_tensor("b1", [G, H], F32, kind="ExternalInput").ap()
    b2_d = nc.dram_tensor("b2", [G, H], F32, kind="ExternalInput").ap()
    b3_d = nc.dram_tensor("b3", [G, C], F32, kind="ExternalInput").ap()
    out_d = [nc.dram_tensor(f"outb{g}", [T, C, S], F32,
                            kind="ExternalOutput").ap()
             for g, (T, S) in enumerate(prof)]

    # flat block list: (g, t) in execution order
    blocks = [(g, t) for g, (T, S) in enumerate(prof) for t in range(T)]
    NB = len(blocks)

    with TileContext(nc) as tc:
        with (
            tc.tile_pool(name="w", bufs=2) as wpool,
            tc.tile_pool(name="x", bufs=3) as xpool,
            tc.tile_pool(name="h1", bufs=3) as h1pool,
            tc.tile_pool(name="h2", bufs=1) as h2pool,
            tc.tile_pool(name="o", bufs=2) as opool,
            tc.tile_pool(name="ps", bufs=8, space="PSUM") as pspool,
        ):
            def emit_weights(g, startup=False):
                # All weight streams ride sync, in consumption order (W1
                # chunks first — the startup-critical stream — then
                # W2/W3), so W2 never steals HBM bandwidth from W1 during
                # the exposed startup window. Biases on scalar. W2/W3/b
                # use 3 bufs so a third set never blocks on the first
                # set's buffer lifetime.
                w1ch = []
                for k in range(KO1):
                    wt = wpool.tile([128, H], BF16, tag=f"w1k{k}")
                    nc.sync.dma_start(wt[:], w1_d[g, :, k, :])
                    w1ch.append(wt)

                b1sb = wpool.tile([128, M1], F32, tag="b1", bufs=3)
                nc.scalar.dma_start(
                    b1sb[:], b1_d[g].rearrange("(m p) -> p m", p=128))
                b2sb = wpool.tile([128, M1], F32, tag="b2", bufs=3)
                nc.scalar.dma_start(
                    b2sb[:], b2_d[g].rearrange("(m p) -> p m", p=128))
                b3sb = wpool.tile([C, 1], F32, tag="b3", bufs=3)
                nc.scalar.dma_start(b3sb[:], b3_d[g][:, None])
                w2ch = []
                for k in range(KO2):
                    wt = wpool.tile([128, H], BF16, tag=f"w2k{k}", bufs=3)
                    nc.sync.dma_start(wt[:], w2_d[g, :, k, :])
                    w2ch.append(wt)
                w3sb = wpool.tile([128, KO2, C], BF16, tag="w3", bufs=3)
                nc.sync.dma_start(w3sb[:], w3_d[g])

                return dict(
                    w1m=lambda k, m: w1ch[k][:, m * 128:(m + 1) * 128],
                    w2=lambda k: w2ch[k][:], w3=w3sb,
                    b1=b1sb, b2=b2sb, b3=b3sb)

            def emit_x(b):
                g, t = blocks[b]
                S = prof[g][1]
                xsb = xpool.tile([128, KO1, S], BF16, tag="x")
                nc.scalar.dma_start(xsb[:], x_d[g][t])
                return xsb

            def emit_L1(b, W, xsb, kouter=False):
                g, _ = blocks[b]
                S = prof[g][1]
                h1sb = h1pool.tile([128, KO2, S], BF16, tag="h1")
                if kouter:
                    # All 8 PSUM banks accumulate in parallel; each W1
                    # chunk is fully consumed on arrival (startup mode).
                    pss = [pspool.tile([128, S], F32, tag="ps",
                                       name=f"ps_ko{m}")
                           for m in range(M1)]
                    for k in range(KO1):
                        for m in range(M1):
                            nc.tensor.matmul(
                                pss[m][:], W["w1m"](k, m), xsb[:, k, :],
                                start=(k == 0), stop=(k == KO1 - 1))
                    for m in range(M1):
                        nc.vector.tensor_scalar(
                            h1sb[:, m, :], pss[m][:], W["b1"][:, m:m + 1],
                            0.0, mybir.AluOpType.add, mybir.AluOpType.max)
                    return h1sb
                for m in range(M1):
                    ps = pspool.tile([128, S], F32, tag="ps",
                                     name=f"ps_{b}_{m}")
                    for k in range(KO1):
                        nc.tensor.matmul(
                            ps[:], W["w1m"](k, m), xsb[:, k, :],
                            start=(k == 0), stop=(k == KO1 - 1))
                    nc.vector.tensor_scalar(
                        h1sb[:, m, :], ps[:], W["b1"][:, m:m + 1], 0.0,
                        mybir.AluOpType.add, mybir.AluOpType.max)
                return h1sb

            def emit_L23(b, W, h1sb):
                g, t = blocks[b]
                S = prof[g][1]
                h2sb = h2pool.tile([128, KO2, S], BF16, tag="h2")
                for m in range(M1):
                    ps = pspool.tile([128, S], F32, tag="ps",
                                     name=f"ps2_{b}_{m}")
                    for k in range(KO2):
                        nc.tensor.matmul(
                            ps[:], W["w2"](k)[:, m * 128:(m + 1) * 128],
                            h1sb[:, k, :],
                            start=(k == 0), stop=(k == KO2 - 1))
                    nc.scalar.activation(
                        h2sb[:, m, :], ps[:], relu, bias=W["b2"][:, m:m + 1])
                ps3 = pspool.tile([128, S], F32, tag="ps",
                                  name=f"ps3_{b}")
                for k in range(KO2):
                    nc.tensor.matmul(
                        ps3[:C, :], W["w3"][:, k, :], h2sb[:, k, :],
                        start=(k == 0), stop=(k == KO2 - 1))
                osb = opool.tile([C, S], F32, tag="o")
                nc.scalar.activation(
                    osb[:], ps3[:C, :], ident, bias=W["b3"][:, 0:1])
                nc.gpsimd.dma_start(out_d[g][t], osb[:])

            # Software pipeline, depth 2: L1 of blocks b+1/b+2 are
            # emitted before L2/L3 of block b, so weight-set DMAs and
            # ACT latency never drain the PE.
            Ws = {}
            h1 = {}
            xpre = {}

            def emit_front(b):
                g = blocks[b][0]
                if g not in Ws:
                    Ws[g] = emit_weights(g)
                h1[b] = emit_L1(b, Ws[g], xpre.pop(b) if b in xpre
                                else emit_x(b))

            # Startup: x0 chunks lead the scalar ring (k0 alone so the
            # first matmul can fire, then pairs), set-0 W1 streams on
            # sync, W2/W3 on gpsimd, x1..x4 follow on scalar. Block 0's
            # L1 runs k-outer so every chunk is consumed on arrival.
            g0 = blocks[0][0]
            T0, S0 = prof[0]
            if T0 >= 3:
                xsb0 = xpool.tile([128, KO1, S0], BF16, tag="x",
                                  name="x0")
                nc.scalar.dma_start(xsb0[:, 0, :], x_d[0][0, :, 0, :])
                ks = 1
                while ks < KO1:
                    ke = min(ks + 2, KO1)
                    nc.scalar.dma_start(xsb0[:, ks:ke, :],
                                        x_d[0][0, :, ks:ke, :])
                    ks = ke
                Ws[g0] = emit_weights(g0, startup=True)
                xs1 = emit_x(1)
                xs2 = emit_x(2)
                for bb in (3, 4):
                    if bb < NB:
                        g, t = blocks[bb]
                        S = prof[g][1]
                        xp = xpool.tile([128, KO1, S], BF16,
                                        tag="x", name=f"xpre{bb}")
                        nc.scalar.dma_start(xp[:], x_d[g][t])
                        xpre[bb] = xp
                h1[0] = emit_L1(0, Ws[g0], xsb0, kouter=True)
                h1[1] = emit_L1(1, Ws[g0], xs1)
                h1[2] = emit_L1(2, Ws[g0], xs2)
                emitted = 2
            else:
                emit_front(0)
                emitted = 0
            for b in range(NB):
                for nxt in range(emitted + 1, min(b + 3, NB)):
                    emit_front(nxt)
                    emitted = nxt
                if b + 4 < NB and blocks[b + 4][0] not in Ws:
                    Ws[blocks[b + 4][0]] = emit_weights(blocks[b + 4][0])
                emit_L23(b, Ws[blocks[b][0]], h1.pop(b))

    nc.compile()
    _program_cache[key] = nc
    return nc


# ---------------------------------------------------------------- host
def _execute(inputs, trace=False, trace_cores=None):
    graph = np.ascontiguousarray(inputs["graph"], dtype=np.float32)
    state = np.ascontiguousarray(inputs["state"], dtype=np.float32)
    next_state = np.ascontiguousarray(inputs["next_state"], dtype=np.float32)
    W1 = np.ascontiguousarray(inputs["W1"], dtype=np.float32)
    b1 = np.ascontiguousarray(inputs["b1"], dtype=np.float32)
    W2 = np.ascontiguousarray(inputs["W2"], dtype=np.float32)
    b2 = np.ascontiguousarray(inputs["b2"], dtype=np.float32)
    W3 = np.ascontiguousarray(inputs["W3"], dtype=np.float32)
    b3 = np.ascontiguousarray(inputs["b3"], dtype=np.float32)

    B = graph.shape[0]
    NF, IN, H = W1.shape
    C = W3.shape[2]
    assert IN == graph.shape[1] + state.shape[1] + next_state.shape[1]
    assert H % 128 == 0 and C <= 128
    INP = ((IN + 127) // 128) * 128
    KO1 = INP // 128
    KO2 = H // 128

    out_full = np.zeros((B, C), dtype=np.float32)

    # --- route: last active factor per row
    mask = graph[:, :NF] == 1.0
    active = mask.any(axis=1)
    last = (NF - 1) - np.argmax(mask[:, ::-1], axis=1)
    if not active.any():
        return (out_full, None) if trace else out_full

    rows_by_e = [np.nonzero(active & (last == e))[0] for e in range(NF)]
    prof, expert_of = _make_plan([len(r) for r in rows_by_e])
    G = len(prof)

    # --- pack rows into per-core slot blocks
    # rowmap[core][g] : int64 [T_g, S_g], original row id or -1 (pad)
    rowmap = [[np.full((T, S), -1, dtype=np.int64) for (T, S) in prof]
              for _ in range(NCORES)]
    slots_by_e = {}
    for core in range(NCORES):
        for g in range(G):
            slots_by_e.setdefault(expert_of[core][g], []).append((core, g))
    for e in range(NF):
        rows = rows_by_e[e]
        if len(rows) == 0:
            continue
        pos = 0
        for core, g in slots_by_e.get(e, []):
            T, S = prof[g]
            take = min(T * S, len(rows) - pos)
            if take <= 0:
                break
            flat = rowmap[core][g].reshape(-1)
            flat[:take] = rows[pos:pos + take]
            pos += take
        assert pos == len(rows), f"expert {e} rows not fully packed"

    # --- build per-core inputs
    x = np.concatenate([graph, state, next_state], axis=1)  # [B, IN]
    if INP != IN:
        x = np.concatenate([x, np.zeros((B, INP - IN), np.float32)], axis=1)
    xpad = np.concatenate([x, np.zeros((1, INP), np.float32)], axis=0)
    W1p = np.zeros((NF, INP, H), np.float32)
    W1p[:, :IN] = W1

    # Partition-major device layouts: [.., 128, KO, free] so every DMA
    # line is one contiguous run per partition.
    W1pm = np.ascontiguousarray(
        W1p.reshape(NF, KO1, 128, H).transpose(0, 2, 1, 3)).astype(bfloat16)
    W2pm = np.ascontiguousarray(
        W2.reshape(NF, KO2, 128, H).transpose(0, 2, 1, 3)).astype(bfloat16)
    W3pm = np.ascontiguousarray(
        W3.reshape(NF, KO2, 128, C).transpose(0, 2, 1, 3)).astype(bfloat16)
    in_maps = []
    for core in range(NCORES):
        es = expert_of[core]
        im = {
            "w1": W1pm[es],
            "w2": W2pm[es],
            "w3": W3pm[es],
            "b1": np.ascontiguousarray(b1[es]),
            "b2": np.ascontiguousarray(b2[es]),
            "b3": np.ascontiguousarray(b3[es]),
        }
        for g, (T, S) in enumerate(prof):
            xb = xpad[rowmap[core][g].reshape(-1)]  # [T*S, INP]; -1 -> 0row
            im[f"xb{g}"] = np.ascontiguousarray(
                xb.reshape(T, S, KO1, 128).transpose(0, 3, 2, 1)
            ).astype(bfloat16)
        in_maps.append(im)

    nc = _build_program(tuple(prof), KO1, KO2, H, C)
    kwargs = {}
    if trace:
        kwargs = dict(trace=True,
                      trace_cores=trace_cores or list(range(NCORES)))
    res = run_bass_kernel_spmd(nc, in_maps, list(range(NCORES)), **kwargs)

    # --- scatter back
    for core in range(NCORES):
        for g, (T, S) in enumerate(prof):
            ob = np.asarray(res.results[core][f"outb{g}"])  # [T, C, S]
            rows = ob.transpose(0, 2, 1).reshape(T * S, C)
            ids = rowmap[core][g].reshape(-1)
            valid = ids >= 0
            out_full[ids[valid]] = rows[valid]

    return (out_full, res) if trace else out_full


def kernel(**inputs):
    return _execute(inputs)
